# revision 50
# baseline (speedup 1.0000x reference)
"""DiffFOOOF loss on 8 NeuronCores — pure data parallelism over batch.

Each core processes B/8 = 1024 rows; the host sums the per-core /
per-partition partials into the final scalar loss (all heavy math is
on-device; the host only adds a few hundred f32 partials).

Design (vs the 87µs v1 baseline, from perfetto trace analysis; now
~62-70µs, which is the per-core HBM roofline: 16.8MB/358GBps = 47µs of
mandatory reads + ~8µs NEFF startup + ~6µs tail + teardown):
  * v1 was compute-bound: DVE 81µs / ACT 73µs busy vs a ~50µs DMA
    window, and all 16 HWDGE loads completed near-simultaneously at
    ~60µs (queued instructions drain round-robin), so compute started
    at 25µs and tailed to 100µs.
  * Loads go through nc.gpsimd (SWDGE) with f32->bf16 cast in flight:
    each dma_start self-splits across all 16 SDMA engines, so chunks
    complete sequentially and compute streams right behind the DMA.
    Megatile "(p g) f" layout gives 16KB-contiguous per-partition
    descriptors. Chunk sizes [1,2,2,1,1,.5,.5]F keep the first arrival
    early and the last-chunk tail short.
  * Huber: huber(e) = 0.5*clamp(e,-1,1)^2 + relu(e-1) + relu(-e-1).
    Per chunk: DVE does subtract (TT, 2 elem/cyc bf16) + clamp + min
    preps (TS single-input ops hit the 4 elem/cyc packed mode; 2-input
    STT and any DVE reduce run at 1x — avoid them on the big path).
    ScalarE (dtype-independent 1 elem/cyc) does Square(clamp)+accum and
    Relu(e-1)+accum. The third sum rides the idle PE: -ones^T x
    min(e,-1) slices accumulate into one [1,512] PSUM register (single
    lhsT tile: swapping ldweights inside one PSUM accumulation group
    intermittently crashed the exec unit). The last THREE chunks' relu
    sums are DVE fused tensor_scalar reduces: when HBM contention
    compresses late arrivals, ScalarE otherwise becomes a 15µs serial
    tail chain (Square+Relu per chunk at ~1ns/elem, dtype-independent).
  * Greedy peak matching is reformulated as packed integer argmin:
    packed = round(4096*|gt-cf|) + i/8 (+2^23 once used). One TRmin
    gives value+argmin (first-occurrence tiebreak = fraction bits);
    is_equal(dm, min+penalty) recovers the one-hot hit row directly.
    5 DVE ops per scan step, epilogue sums via fused accum_out, and the
    whole chain depends only on the small PACK dma, so the scheduler
    hides it entirely inside the initial DMA ramp.
  * All small tensors are pre-packed host-side into one [128, 657] f32
    array (pure layout transform) -> a single early DMA.
"""

import numpy as np

import concourse.bass as bass
import concourse.tile as tile
from concourse import bacc, mybir
from concourse.bass_utils import run_bass_kernel_spmd

f32 = mybir.dt.float32
bf16 = mybir.dt.bfloat16
Alu = mybir.AluOpType
Act = mybir.ActivationFunctionType
X = mybir.AxisListType.X

N_CORES = 8
B, F, K = 8192, 2048, 6
BS = B // N_CORES        # rows per core
P = 128                  # partitions
G = BS // P              # row-groups per partition (8)

Q23 = float(2.0 ** 23)
Q25 = float(2.0 ** 25)
QS = 4096.0              # dist quantization scale (1/4096 granularity)

# PACK column layout (f32, [128, PACKC]); row r = p*G + g
PC_V = 0                  # cfs|amps|bws, col = v*48 + g*6 + i
PC_GT = 144               # gt_cfs|gt_amps|gt_bws
PC_M = 288                # peak_mask, col = g*6 + j
PC_EXP = 336              # exponent|offset, col = g (16 cols)
PC_GEXP = 352             # gt_exponent|gt_offset (16 cols)
PC_IOTA = 368             # i * 0.125 at col g*36 + j*6 + i  (288 cols)
PC_ONES = 656             # 1.0
PACKC = 657

# ACC column layout (each col later summed over partitions)
# huber(e) = 0.5*clamp(e,-1,1)^2 + relu(e-1) + relu(-e-1).
# Sum clamp^2 and Sum relu(e-1) are ScalarE accum columns; the third
# sum comes from PE: -ones^T x min(e,-1) -> [1,512] PSUM partials
# shipped to the host (Sum relu(-e-1) = pe_sum - N).
# The LAST chunk's relu(e-1) is a DVE fused reduce of max(e,1) into its
# C_RP column (so ScalarE is off the critical tail); host subtracts N/8.
CHUNK_COLS = [2048, 4096, 4096, 2048, 2048, 1024, 1024]  # sum = G*F
NCH = len(CHUNK_COLS)
C_C2, C_RP = 0, NCH                 # per-chunk cols
C_PK, C_AMPS, C_BW2, C_AP, C_UMN, C_UMD, C_MASK = (
    2 * NCH, 2 * NCH + 1, 2 * NCH + 2, 2 * NCH + 3, 2 * NCH + 4,
    2 * NCH + 5, 2 * NCH + 6)
ACC_COLS = 2 * NCH + 7
N_DVE_RELU = 3            # trailing chunks whose relu sum is a DVE reduce
LAST_FRAC = sum(CHUNK_COLS[-N_DVE_RELU:]) / float(G * F)  # host N-correction


def build_nc():
    from contextlib import ExitStack

    nc = bacc.Bacc("TRN2", target_bir_lowering=False, debug=False,
                   num_devices=N_CORES)
    pred = nc.dram_tensor("pred_psd", [BS, F], f32, kind="ExternalInput")
    true = nc.dram_tensor("true_psd", [BS, F], f32, kind="ExternalInput")
    pack = nc.dram_tensor("pack", [P, PACKC], f32, kind="ExternalInput")
    out_d = nc.dram_tensor("out", [P, ACC_COLS], f32, kind="ExternalOutput")
    out2_d = nc.dram_tensor("out2", [1, 512], f32, kind="ExternalOutput")

    pred_r = pred[:, :].rearrange("(p g) f -> p (g f)", p=P)
    true_r = true[:, :].rearrange("(p g) f -> p (g f)", p=P)

    with tile.TileContext(nc) as tc, ExitStack() as ctx:
        sp = ctx.enter_context(tc.tile_pool(name="small", bufs=1))
        mp = ctx.enter_context(tc.tile_pool(name="match", bufs=1))
        bigp = ctx.enter_context(tc.tile_pool(name="big", bufs=1))
        epool = ctx.enter_context(tc.tile_pool(name="e", bufs=2))
        apool = ctx.enter_context(tc.tile_pool(name="a", bufs=2))
        spool = ctx.enter_context(tc.tile_pool(name="s", bufs=2))
        dpool = ctx.enter_context(tc.tile_pool(name="dump", bufs=2))
        psp = ctx.enter_context(tc.tile_pool(name="ps", bufs=1, space="PSUM"))

        # ---------------- DMAs: PACK first, then big chunks ------------
        PK = sp.tile([P, PACKC], f32)
        nc.gpsimd.dma_start(out=PK[:], in_=pack[:, :])

        pm = bigp.tile([P, G * F], bf16, name="pred_mega")
        tm = bigp.tile([P, G * F], bf16, name="true_mega")
        c0 = 0
        bounds = []
        for w in CHUNK_COLS:
            lo, hi = c0, c0 + w
            nc.gpsimd.dma_start(out=pm[:, lo:hi], in_=pred_r[:, lo:hi])
            nc.gpsimd.dma_start(out=tm[:, lo:hi], in_=true_r[:, lo:hi])
            bounds.append((lo, hi))
            c0 += w

        ACC = sp.tile([P, ACC_COLS], f32)
        nc.vector.memset(ACC[:], 0.0)

        # ---------------- matching (depends only on PK) ----------------
        cfs3 = PK[:, PC_V:PC_V + 48].rearrange("p (g i) -> p g i", i=K)
        gtc3 = PK[:, PC_GT:PC_GT + 48].rearrange("p (g j) -> p g j", j=K)
        M = PK[:, PC_M:PC_M + 48]

        dist = mp.tile([P, G * K * K], f32)       # col = g*36 + j*6 + i
        dist4 = dist[:].rearrange("p (g j i) -> p g j i", j=K, i=K)
        nc.vector.tensor_tensor(
            out=dist4,
            in0=gtc3.unsqueeze(3).to_broadcast([P, G, K, K]),
            in1=cfs3.unsqueeze(2).to_broadcast([P, G, K, K]),
            op=Alu.subtract)
        adist = mp.tile([P, G * K * K], f32)
        nc.vector.scalar_tensor_tensor(out=adist[:], in0=dist[:], scalar=-1.0,
                                       in1=dist[:], op0=Alu.mult, op1=Alu.max)
        # tq = 4096*|d| + 2^23  (rounds to integer); ScalarE, off the DVE
        tq = mp.tile([P, G * K * K], f32)
        nc.scalar.activation(out=tq[:], in_=adist[:], func=Act.Copy,
                             scale=QS, bias=Q23)
        # packed = (tq - 2^23) + i/8   (exact: int + fraction tiebreak)
        packed = mp.tile([P, G * K * K], f32)
        packed4 = packed[:].rearrange("p (g j i) -> p g j i", j=K, i=K)
        nc.vector.scalar_tensor_tensor(
            out=packed[:], in0=tq[:],
            scalar=Q23, in1=PK[:, PC_IOTA:PC_IOTA + G * K * K],
            op0=Alu.subtract, op1=Alu.add)
        # penal[g,j] = (1-mask)*2^25 : inactive GT slots can never match
        penal = mp.tile([P, G * K], f32)
        nc.scalar.activation(out=penal[:], in_=M, func=Act.Copy,
                             scale=-Q25, bias=Q25)
        penal3 = penal[:].rearrange("p (g j) -> p g j", j=K)

        H = mp.tile([P, G * K * K], f32)          # one-hot hits [g, j, i]
        H4 = H[:].rearrange("p (g j i) -> p g j i", j=K, i=K)
        used_t = []
        for j in range(K + 1):
            uj = mp.tile([P, G * K], f32, tag=f"used{j}", name=f"used{j}")
            used_t.append(uj)
        nc.vector.memset(used_t[0][:], 0.0)

        for j in range(K):
            u3 = used_t[j][:].rearrange("p (g i) -> p g i", i=K)
            dm = mp.tile([P, G * K], f32, tag="dm", name=f"dm{j}")
            dm3 = dm[:].rearrange("p (g i) -> p g i", i=K)
            nc.vector.tensor_tensor(out=dm3, in0=packed4[:, :, j, :],
                                    in1=u3, op=Alu.add)
            bm = mp.tile([P, G], f32, tag="bm", name=f"bm{j}")
            nc.vector.tensor_reduce(out=bm[:], in_=dm3, axis=X, op=Alu.min)
            bmm = mp.tile([P, G], f32, tag="bmm", name=f"bmm{j}")
            nc.vector.tensor_tensor(out=bmm[:], in0=bm[:],
                                    in1=penal3[:, :, j], op=Alu.add)
            hj = H4[:, :, j, :]
            nc.vector.tensor_tensor(
                out=hj, in0=dm3,
                in1=bmm[:].unsqueeze(2).to_broadcast([P, G, K]),
                op=Alu.is_equal)
            un3 = used_t[j + 1][:].rearrange("p (g i) -> p g i", i=K)
            nc.vector.scalar_tensor_tensor(out=un3, in0=hj, scalar=Q23,
                                           in1=u3, op0=Alu.mult, op1=Alu.add)

        # ---------------- matching epilogue ----------------------------
        # batched gather: Gt[p,v,g,j] = sum_i H[p,g,j,i] * V[p,v,g,i]
        Vv = PK[:, PC_V:PC_V + 144].rearrange("p (v g i) -> p v g i", v=3, i=K)
        gm = mp.tile([P, 3 * G * K * K], f32)
        gm5 = gm[:].rearrange("p (v g j i) -> p v g j i", v=3, j=K, i=K)
        nc.vector.tensor_tensor(
            out=gm5,
            in0=Vv.unsqueeze(3).to_broadcast([P, 3, G, K, K]),
            in1=H4.unsqueeze(1).to_broadcast([P, 3, G, K, K]),
            op=Alu.mult)
        Gt = mp.tile([P, 3 * G * K], f32)         # col = v*48 + g*6 + j
        Gt4 = Gt[:].rearrange("p (v g j) -> p v g j", v=3, j=K)
        nc.vector.tensor_reduce(out=Gt4, in_=gm5, axis=X, op=Alu.add)
        # l_peaks: GT inputs are pre-masked (zero at inactive), H is
        # mask-gated, so D = Gt - GT is already the masked difference.
        D = mp.tile([P, 3 * G * K], f32)
        nc.vector.tensor_tensor(out=D[:], in0=Gt[:],
                                in1=PK[:, PC_GT:PC_GT + 144], op=Alu.subtract)
        dpk = mp.tile([P, 3 * G * K], f32)
        nc.vector.scalar_tensor_tensor(out=dpk[:], in0=D[:], scalar=1.0,
                                       in1=D[:], op0=Alu.mult, op1=Alu.mult,
                                       accum_out=ACC[:, C_PK:C_PK + 1])

        du = mp.tile([P, G * K], f32, tag="du")
        nc.vector.tensor_scalar(out=du[:], in0=PK[:, PC_V + 48:PC_V + 96],
                                scalar1=1.0, scalar2=0.0, op0=Alu.mult,
                                op1=Alu.add,
                                accum_out=ACC[:, C_AMPS:C_AMPS + 1])
        rb = mp.tile([P, G * K], f32)
        nc.vector.tensor_scalar(out=rb[:], in0=PK[:, PC_V + 96:PC_V + 144],
                                scalar1=4.0, scalar2=0.0,
                                op0=Alu.subtract, op1=Alu.max)
        du2 = mp.tile([P, G * K], f32, tag="du")
        nc.vector.scalar_tensor_tensor(out=du2[:], in0=rb[:], scalar=1.0,
                                       in1=rb[:], op0=Alu.mult, op1=Alu.mult,
                                       accum_out=ACC[:, C_BW2:C_BW2 + 1])
        dEO = mp.tile([P, 2 * G], f32)
        nc.vector.tensor_tensor(out=dEO[:], in0=PK[:, PC_EXP:PC_EXP + 2 * G],
                                in1=PK[:, PC_GEXP:PC_GEXP + 2 * G],
                                op=Alu.subtract)
        dg = mp.tile([P, 2 * G], f32, tag="dg")
        nc.vector.scalar_tensor_tensor(out=dg[:], in0=dEO[:], scalar=1.0,
                                       in1=dEO[:], op0=Alu.mult, op1=Alu.mult,
                                       accum_out=ACC[:, C_AP:C_AP + 1])
        # unmatched: used in {0, 2^23} -> unm1 = used * -2^-23 = unm - 1.
        # (tensor_scalar with accum_out treats op1 as the REDUCE op.)
        # Host corrects: SUD = B*K + sum(unm1), SUN = SA + sum(unm1*amps).
        unm1 = mp.tile([P, G * K], f32)
        nc.vector.tensor_scalar(out=unm1[:], in0=used_t[K][:],
                                scalar1=-(2.0 ** -23), scalar2=0.0,
                                op0=Alu.mult, op1=Alu.add,
                                accum_out=ACC[:, C_UMD:C_UMD + 1])
        du3 = mp.tile([P, G * K], f32, tag="du")
        nc.vector.scalar_tensor_tensor(out=du3[:], in0=unm1[:], scalar=1.0,
                                       in1=PK[:, PC_V + 48:PC_V + 96],
                                       op0=Alu.mult, op1=Alu.mult,
                                       accum_out=ACC[:, C_UMN:C_UMN + 1])
        du4 = mp.tile([P, G * K], f32, tag="du")
        nc.vector.tensor_scalar(out=du4[:], in0=M, scalar1=1.0, scalar2=0.0,
                                op0=Alu.mult, op1=Alu.add,
                                accum_out=ACC[:, C_MASK:C_MASK + 1])

        # ---------------- huber big loop (per load chunk) ---------------
        # DVE: subtract + two 1-input TS ops (4x bf16 packed mode).
        # ScalarE: Square(clamp)+accum and Relu(e-1)+accum.
        # PE: ones^T x min(e,-1) 512-col slices accumulated into PSUM.
        negones_bf = sp.tile([P, 1], bf16)
        nc.vector.memset(negones_bf[:], -1.0)
        neg1 = sp.tile([P, 1], f32)
        nc.vector.memset(neg1[:], -1.0)
        pst = psp.tile([P, 512], f32)
        # PE accumulates sum(-m) over all chunks (single lhsT tile:
        # swapping ldweights inside one PSUM accumulation group was
        # observed to intermittently crash the exec unit).
        n_sl = sum((hi - lo) // 512 for lo, hi in bounds)
        sl_i = 0
        last = len(bounds) - 1
        for ci, (lo, hi) in enumerate(bounds):
            W = hi - lo
            e = epool.tile([P, W], bf16, tag="e", name=f"e{ci}")
            nc.vector.tensor_tensor(out=e[:], in0=pm[:, lo:hi],
                                    in1=tm[:, lo:hi], op=Alu.subtract)
            c = apool.tile([P, W], bf16, tag="c", name=f"c{ci}")
            nc.vector.tensor_scalar(out=c[:], in0=e[:], scalar1=-1.0,
                                    scalar2=1.0, op0=Alu.max, op1=Alu.min)
            m = spool.tile([P, W], bf16, tag="m", name=f"m{ci}")
            nc.vector.tensor_scalar(out=m[:], in0=e[:], scalar1=-1.0,
                                    scalar2=None, op0=Alu.min)
            d3 = dpool.tile([P, W], bf16, tag="d3", name=f"d3_{ci}")
            nc.scalar.activation(out=d3[:], in_=c[:], func=Act.Square,
                                 accum_out=ACC[:, C_C2 + ci:C_C2 + ci + 1])
            if ci > last - N_DVE_RELU:
                # DVE fused reduce: sum max(e,1) (host subtracts N*LAST_FRAC)
                d4 = dpool.tile([P, W], bf16, tag="d4", name=f"d4_{ci}")
                nc.vector.tensor_scalar(
                    out=d4[:], in0=e[:], scalar1=1.0, scalar2=0.0,
                    op0=Alu.max, op1=Alu.add,
                    accum_out=ACC[:, C_RP + ci:C_RP + ci + 1])
            else:
                d4 = dpool.tile([P, W], bf16, tag="d4", name=f"d4_{ci}")
                nc.scalar.activation(out=d4[:], in_=e[:], func=Act.Relu,
                                     bias=neg1[:],
                                     accum_out=ACC[:, C_RP + ci:C_RP + ci + 1])
            for sl in range(W // 512):
                nc.tensor.matmul(out=pst[0:1, :], lhsT=negones_bf[:],
                                 rhs=m[:, sl * 512:(sl + 1) * 512],
                                 start=(sl_i == 0), stop=(sl_i == n_sl - 1))
                sl_i += 1

        # ---------------- store (host does the partition sums) ---------
        nc.sync.dma_start(out=out_d[:, :], in_=ACC[:])
        msb = sp.tile([1, 512], f32)
        nc.scalar.copy(out=msb[:], in_=pst[0:1, :])
        nc.sync.dma_start(out=out2_d[:, :], in_=msb[:])
    nc.compile()
    return nc


_NC_CACHE = None


def _get_nc():
    global _NC_CACHE
    if _NC_CACHE is None:
        _NC_CACHE = build_nc()
    return _NC_CACHE


def _make_pack(inputs, lo, hi):
    """[128, PACKC] f32; row r = p*G + g maps to input row lo + r."""
    pk = np.empty((P, PACKC), dtype=np.float32)

    def blk(name):
        return np.ascontiguousarray(inputs[name][lo:hi]).reshape(P, G * K)

    pk[:, PC_V:PC_V + 48] = blk("cfs")
    pk[:, PC_V + 48:PC_V + 96] = blk("amps")
    pk[:, PC_V + 96:PC_V + 144] = blk("bws")
    pk[:, PC_GT:PC_GT + 48] = blk("gt_cfs")
    pk[:, PC_GT + 48:PC_GT + 96] = blk("gt_amps")
    pk[:, PC_GT + 96:PC_GT + 144] = blk("gt_bws")
    pk[:, PC_M:PC_M + 48] = blk("peak_mask")
    pk[:, PC_EXP:PC_EXP + G] = inputs["exponent"][lo:hi].reshape(P, G)
    pk[:, PC_EXP + G:PC_EXP + 2 * G] = inputs["offset"][lo:hi].reshape(P, G)
    pk[:, PC_GEXP:PC_GEXP + G] = inputs["gt_exponent"][lo:hi].reshape(P, G)
    pk[:, PC_GEXP + G:PC_GEXP + 2 * G] = inputs["gt_offset"][lo:hi].reshape(P, G)
    pk[:, PC_IOTA:PC_IOTA + G * K * K] = np.tile(
        (np.arange(K, dtype=np.float32) * 0.125), G * K)[None, :]
    pk[:, PC_ONES] = 1.0
    return pk


def combine(parts, pe_sum):
    """parts: [n_cores, ACC_COLS] f64.

    pe_sum = sum(-min(e,-1)) over ALL = sum relu(-e-1) + N.
    The last N_DVE_RELU C_RP cols hold sum max(e,1) = sum relu(e-1) + N*frac.
    """
    s = parts.sum(axis=0)
    n_big = float(B) * F
    S_c2 = s[C_C2:C_C2 + NCH].sum()    # sum clamp(e,-1,1)^2
    S_rp = s[C_RP:C_RP + NCH].sum() - n_big * LAST_FRAC
    huber_sum = 0.5 * S_c2 + S_rp + (pe_sum - n_big)
    l_recon = huber_sum / n_big
    SA = s[C_AMPS]
    l_sparse = SA / (B * K)
    l_bw = s[C_BW2] / (B * K)
    l_ap = s[C_AP] / B
    l_peaks = s[C_PK] / max(s[C_MASK], 1.0)
    SUN = SA + s[C_UMN]                # sum unm*amps (device has (unm-1)*amps)
    SUD = float(B) * K + s[C_UMD]      # sum unm     (device has unm-1 summed)
    l_um = SUN / max(SUD, 1.0)
    return (l_recon + 0.1 * l_sparse + 0.05 * l_bw + 0.5 * l_ap
            + 0.3 * l_peaks + 0.1 * l_um)


def run(inputs, **spmd_kwargs):
    nc = _get_nc()
    in_maps = []
    for c in range(N_CORES):
        lo, hi = c * BS, (c + 1) * BS
        in_maps.append({
            "pred_psd": np.ascontiguousarray(inputs["pred_psd"][lo:hi]),
            "true_psd": np.ascontiguousarray(inputs["true_psd"][lo:hi]),
            "pack": _make_pack(inputs, lo, hi),
        })
    res = run_bass_kernel_spmd(nc, in_maps, list(range(N_CORES)), **spmd_kwargs)
    parts = np.stack([r["out"].astype(np.float64).sum(axis=0)
                      for r in res.results])
    pe_sum = sum(float(r["out2"].astype(np.float64).sum()) for r in res.results)
    return np.float32(combine(parts, pe_sum)), res


def kernel(**inputs):
    out, _ = run(inputs)
    return out


# revision 51
# speedup vs baseline: 1.0231x; 1.0231x over previous
"""DiffFOOOF loss on 8 NeuronCores — pure data parallelism over batch.

Each core processes B/8 = 1024 rows; the host sums the per-core /
per-partition partials into the final scalar loss (all heavy math is
on-device; the host only adds a few hundred f32 partials).

Design (vs the 87µs v1 baseline, from perfetto trace analysis; now
~62-70µs, which is the per-core HBM roofline: 16.8MB/358GBps = 47µs of
mandatory reads + ~8µs NEFF startup + ~6µs tail + teardown):
  * v1 was compute-bound: DVE 81µs / ACT 73µs busy vs a ~50µs DMA
    window, and all 16 HWDGE loads completed near-simultaneously at
    ~60µs (queued instructions drain round-robin), so compute started
    at 25µs and tailed to 100µs.
  * Loads go through nc.gpsimd (SWDGE) with f32->bf16 cast in flight:
    each dma_start self-splits across all 16 SDMA engines, so chunks
    complete sequentially and compute streams right behind the DMA.
    Megatile "(p g) f" layout gives 16KB-contiguous per-partition
    descriptors. Chunk sizes [1,2,2,1,1,.5,.5]F keep the first arrival
    early and the last-chunk tail short.
  * Huber: huber(e) = 0.5*clamp(e,-1,1)^2 + relu(e-1) + relu(-e-1).
    Per chunk: DVE does subtract (TT, 2 elem/cyc bf16) + clamp + min
    preps (TS single-input ops hit the 4 elem/cyc packed mode; 2-input
    STT and any DVE reduce run at 1x — avoid them on the big path).
    ScalarE (dtype-independent 1 elem/cyc) does Square(clamp)+accum and
    Relu(e-1)+accum. The third sum rides the idle PE: -ones^T x
    min(e,-1) slices accumulate into one [1,512] PSUM register (single
    lhsT tile: swapping ldweights inside one PSUM accumulation group
    intermittently crashed the exec unit). The last THREE chunks' relu
    sums are DVE fused tensor_scalar reduces: when HBM contention
    compresses late arrivals, ScalarE otherwise becomes a 15µs serial
    tail chain (Square+Relu per chunk at ~1ns/elem, dtype-independent).
  * Greedy peak matching is reformulated as packed integer argmin:
    packed = round(4096*|gt-cf|) + i/8 (+2^23 once used). One TRmin
    gives value+argmin (first-occurrence tiebreak = fraction bits);
    is_equal(dm, min+penalty) recovers the one-hot hit row directly.
    5 DVE ops per scan step, epilogue sums via fused accum_out, and the
    whole chain depends only on the small PACK dma, so the scheduler
    hides it entirely inside the initial DMA ramp.
  * All small tensors are pre-packed host-side into one [128, 657] f32
    array (pure layout transform) -> a single early DMA.
"""

import numpy as np

import concourse.bass as bass
import concourse.tile as tile
from concourse import bacc, mybir
from concourse.bass_utils import run_bass_kernel_spmd

f32 = mybir.dt.float32
bf16 = mybir.dt.bfloat16
Alu = mybir.AluOpType
Act = mybir.ActivationFunctionType
X = mybir.AxisListType.X

N_CORES = 8
B, F, K = 8192, 2048, 6
BS = B // N_CORES        # rows per core
P = 128                  # partitions
G = BS // P              # row-groups per partition (8)

Q23 = float(2.0 ** 23)
Q25 = float(2.0 ** 25)
QS = 4096.0              # dist quantization scale (1/4096 granularity)

# PACK column layout (f32, [128, PACKC]); row r = p*G + g
PC_V = 0                  # cfs|amps|bws, col = v*48 + g*6 + i
PC_GT = 144               # gt_cfs|gt_amps|gt_bws
PC_M = 288                # peak_mask, col = g*6 + j
PC_EXP = 336              # exponent|offset, col = g (16 cols)
PC_GEXP = 352             # gt_exponent|gt_offset (16 cols)
PC_IOTA = 368             # i * 0.125 at col g*36 + j*6 + i  (288 cols)
PC_ONES = 656             # 1.0
PACKC = 657

# ACC column layout (each col later summed over partitions)
# huber(e) = 0.5*clamp(e,-1,1)^2 + relu(e-1) + relu(-e-1).
# Sum clamp^2 and Sum relu(e-1) are ScalarE accum columns; the third
# sum comes from PE: -ones^T x min(e,-1) -> [1,512] PSUM partials
# shipped to the host (Sum relu(-e-1) = pe_sum - N).
# The LAST chunk's relu(e-1) is a DVE fused reduce of max(e,1) into its
# C_RP column (so ScalarE is off the critical tail); host subtracts N/8.
CHUNK_COLS = [2048, 4096, 4096, 2048, 2048, 1024, 1024]  # sum = G*F
NCH = len(CHUNK_COLS)
C_C2, C_RP = 0, NCH                 # per-chunk cols
C_PK, C_AMPS, C_BW2, C_AP, C_UMN, C_UMD, C_MASK = (
    2 * NCH, 2 * NCH + 1, 2 * NCH + 2, 2 * NCH + 3, 2 * NCH + 4,
    2 * NCH + 5, 2 * NCH + 6)
ACC_COLS = 2 * NCH + 7
N_DVE_RELU = 3            # trailing chunks whose relu sum is a DVE reduce
LAST_FRAC = sum(CHUNK_COLS[-N_DVE_RELU:]) / float(G * F)  # host N-correction


def build_nc():
    from contextlib import ExitStack

    nc = bacc.Bacc("TRN2", target_bir_lowering=False, debug=False,
                   num_devices=N_CORES)
    pred = nc.dram_tensor("pred_psd", [BS, F], f32, kind="ExternalInput")
    true = nc.dram_tensor("true_psd", [BS, F], f32, kind="ExternalInput")
    pack = nc.dram_tensor("pack", [P, PACKC], f32, kind="ExternalInput")
    out_d = nc.dram_tensor("out", [P, ACC_COLS], f32, kind="ExternalOutput")
    out2_d = nc.dram_tensor("out2", [1, 512], f32, kind="ExternalOutput")

    pred_r = pred[:, :].rearrange("(p g) f -> p (g f)", p=P)
    true_r = true[:, :].rearrange("(p g) f -> p (g f)", p=P)

    with tile.TileContext(nc) as tc, ExitStack() as ctx:
        sp = ctx.enter_context(tc.tile_pool(name="small", bufs=1))
        mp = ctx.enter_context(tc.tile_pool(name="match", bufs=1))
        bigp = ctx.enter_context(tc.tile_pool(name="big", bufs=1))
        epool = ctx.enter_context(tc.tile_pool(name="e", bufs=2))
        apool = ctx.enter_context(tc.tile_pool(name="a", bufs=2))
        spool = ctx.enter_context(tc.tile_pool(name="s", bufs=2))
        dpool = ctx.enter_context(tc.tile_pool(name="dump", bufs=2))
        psp = ctx.enter_context(tc.tile_pool(name="ps", bufs=1, space="PSUM"))

        # ---------------- DMAs ----------------------------------------
        # PACK rides the idle sync HWDGE ring (4 col-slices -> 4 SDMA
        # engines) so the Q7/SWDGE queue's first emission is chunk 0 of
        # the big stream — the whole DMA stream starts ~0.7µs earlier
        # and matching still gets PACK with slack.
        PK = sp.tile([P, PACKC], f32)
        for lo4, hi4 in ((0, 165), (165, 330), (330, 495), (495, PACKC)):
            nc.sync.dma_start(out=PK[:, lo4:hi4], in_=pack[:, lo4:hi4])

        pm = bigp.tile([P, G * F], bf16, name="pred_mega")
        tm = bigp.tile([P, G * F], bf16, name="true_mega")
        c0 = 0
        bounds = []
        for w in CHUNK_COLS:
            lo, hi = c0, c0 + w
            nc.gpsimd.dma_start(out=pm[:, lo:hi], in_=pred_r[:, lo:hi])
            nc.gpsimd.dma_start(out=tm[:, lo:hi], in_=true_r[:, lo:hi])
            bounds.append((lo, hi))
            c0 += w

        ACC = sp.tile([P, ACC_COLS], f32)
        nc.vector.memset(ACC[:], 0.0)

        # ---------------- matching (depends only on PK) ----------------
        cfs3 = PK[:, PC_V:PC_V + 48].rearrange("p (g i) -> p g i", i=K)
        gtc3 = PK[:, PC_GT:PC_GT + 48].rearrange("p (g j) -> p g j", j=K)
        M = PK[:, PC_M:PC_M + 48]

        dist = mp.tile([P, G * K * K], f32)       # col = g*36 + j*6 + i
        dist4 = dist[:].rearrange("p (g j i) -> p g j i", j=K, i=K)
        nc.vector.tensor_tensor(
            out=dist4,
            in0=gtc3.unsqueeze(3).to_broadcast([P, G, K, K]),
            in1=cfs3.unsqueeze(2).to_broadcast([P, G, K, K]),
            op=Alu.subtract)
        adist = mp.tile([P, G * K * K], f32)
        nc.vector.scalar_tensor_tensor(out=adist[:], in0=dist[:], scalar=-1.0,
                                       in1=dist[:], op0=Alu.mult, op1=Alu.max)
        # tq = 4096*|d| + 2^23  (rounds to integer); ScalarE, off the DVE
        tq = mp.tile([P, G * K * K], f32)
        nc.scalar.activation(out=tq[:], in_=adist[:], func=Act.Copy,
                             scale=QS, bias=Q23)
        # packed = (tq - 2^23) + i/8   (exact: int + fraction tiebreak)
        packed = mp.tile([P, G * K * K], f32)
        packed4 = packed[:].rearrange("p (g j i) -> p g j i", j=K, i=K)
        nc.vector.scalar_tensor_tensor(
            out=packed[:], in0=tq[:],
            scalar=Q23, in1=PK[:, PC_IOTA:PC_IOTA + G * K * K],
            op0=Alu.subtract, op1=Alu.add)
        # penal[g,j] = (1-mask)*2^25 : inactive GT slots can never match
        penal = mp.tile([P, G * K], f32)
        nc.scalar.activation(out=penal[:], in_=M, func=Act.Copy,
                             scale=-Q25, bias=Q25)
        penal3 = penal[:].rearrange("p (g j) -> p g j", j=K)

        H = mp.tile([P, G * K * K], f32)          # one-hot hits [g, j, i]
        H4 = H[:].rearrange("p (g j i) -> p g j i", j=K, i=K)
        used_t = []
        for j in range(K + 1):
            uj = mp.tile([P, G * K], f32, tag=f"used{j}", name=f"used{j}")
            used_t.append(uj)
        nc.vector.memset(used_t[0][:], 0.0)

        for j in range(K):
            u3 = used_t[j][:].rearrange("p (g i) -> p g i", i=K)
            dm = mp.tile([P, G * K], f32, tag="dm", name=f"dm{j}")
            dm3 = dm[:].rearrange("p (g i) -> p g i", i=K)
            nc.vector.tensor_tensor(out=dm3, in0=packed4[:, :, j, :],
                                    in1=u3, op=Alu.add)
            bm = mp.tile([P, G], f32, tag="bm", name=f"bm{j}")
            nc.vector.tensor_reduce(out=bm[:], in_=dm3, axis=X, op=Alu.min)
            bmm = mp.tile([P, G], f32, tag="bmm", name=f"bmm{j}")
            nc.vector.tensor_tensor(out=bmm[:], in0=bm[:],
                                    in1=penal3[:, :, j], op=Alu.add)
            hj = H4[:, :, j, :]
            nc.vector.tensor_tensor(
                out=hj, in0=dm3,
                in1=bmm[:].unsqueeze(2).to_broadcast([P, G, K]),
                op=Alu.is_equal)
            un3 = used_t[j + 1][:].rearrange("p (g i) -> p g i", i=K)
            nc.vector.scalar_tensor_tensor(out=un3, in0=hj, scalar=Q23,
                                           in1=u3, op0=Alu.mult, op1=Alu.add)

        # ---------------- matching epilogue ----------------------------
        # batched gather: Gt[p,v,g,j] = sum_i H[p,g,j,i] * V[p,v,g,i]
        Vv = PK[:, PC_V:PC_V + 144].rearrange("p (v g i) -> p v g i", v=3, i=K)
        gm = mp.tile([P, 3 * G * K * K], f32)
        gm5 = gm[:].rearrange("p (v g j i) -> p v g j i", v=3, j=K, i=K)
        nc.vector.tensor_tensor(
            out=gm5,
            in0=Vv.unsqueeze(3).to_broadcast([P, 3, G, K, K]),
            in1=H4.unsqueeze(1).to_broadcast([P, 3, G, K, K]),
            op=Alu.mult)
        Gt = mp.tile([P, 3 * G * K], f32)         # col = v*48 + g*6 + j
        Gt4 = Gt[:].rearrange("p (v g j) -> p v g j", v=3, j=K)
        nc.vector.tensor_reduce(out=Gt4, in_=gm5, axis=X, op=Alu.add)
        # l_peaks: GT inputs are pre-masked (zero at inactive), H is
        # mask-gated, so D = Gt - GT is already the masked difference.
        D = mp.tile([P, 3 * G * K], f32)
        nc.vector.tensor_tensor(out=D[:], in0=Gt[:],
                                in1=PK[:, PC_GT:PC_GT + 144], op=Alu.subtract)
        dpk = mp.tile([P, 3 * G * K], f32)
        nc.vector.scalar_tensor_tensor(out=dpk[:], in0=D[:], scalar=1.0,
                                       in1=D[:], op0=Alu.mult, op1=Alu.mult,
                                       accum_out=ACC[:, C_PK:C_PK + 1])

        du = mp.tile([P, G * K], f32, tag="du")
        nc.vector.tensor_scalar(out=du[:], in0=PK[:, PC_V + 48:PC_V + 96],
                                scalar1=1.0, scalar2=0.0, op0=Alu.mult,
                                op1=Alu.add,
                                accum_out=ACC[:, C_AMPS:C_AMPS + 1])
        rb = mp.tile([P, G * K], f32)
        nc.vector.tensor_scalar(out=rb[:], in0=PK[:, PC_V + 96:PC_V + 144],
                                scalar1=4.0, scalar2=0.0,
                                op0=Alu.subtract, op1=Alu.max)
        du2 = mp.tile([P, G * K], f32, tag="du")
        nc.vector.scalar_tensor_tensor(out=du2[:], in0=rb[:], scalar=1.0,
                                       in1=rb[:], op0=Alu.mult, op1=Alu.mult,
                                       accum_out=ACC[:, C_BW2:C_BW2 + 1])
        dEO = mp.tile([P, 2 * G], f32)
        nc.vector.tensor_tensor(out=dEO[:], in0=PK[:, PC_EXP:PC_EXP + 2 * G],
                                in1=PK[:, PC_GEXP:PC_GEXP + 2 * G],
                                op=Alu.subtract)
        dg = mp.tile([P, 2 * G], f32, tag="dg")
        nc.vector.scalar_tensor_tensor(out=dg[:], in0=dEO[:], scalar=1.0,
                                       in1=dEO[:], op0=Alu.mult, op1=Alu.mult,
                                       accum_out=ACC[:, C_AP:C_AP + 1])
        # unmatched: used in {0, 2^23} -> unm1 = used * -2^-23 = unm - 1.
        # (tensor_scalar with accum_out treats op1 as the REDUCE op.)
        # Host corrects: SUD = B*K + sum(unm1), SUN = SA + sum(unm1*amps).
        unm1 = mp.tile([P, G * K], f32)
        nc.vector.tensor_scalar(out=unm1[:], in0=used_t[K][:],
                                scalar1=-(2.0 ** -23), scalar2=0.0,
                                op0=Alu.mult, op1=Alu.add,
                                accum_out=ACC[:, C_UMD:C_UMD + 1])
        du3 = mp.tile([P, G * K], f32, tag="du")
        nc.vector.scalar_tensor_tensor(out=du3[:], in0=unm1[:], scalar=1.0,
                                       in1=PK[:, PC_V + 48:PC_V + 96],
                                       op0=Alu.mult, op1=Alu.mult,
                                       accum_out=ACC[:, C_UMN:C_UMN + 1])
        du4 = mp.tile([P, G * K], f32, tag="du")
        nc.vector.tensor_scalar(out=du4[:], in0=M, scalar1=1.0, scalar2=0.0,
                                op0=Alu.mult, op1=Alu.add,
                                accum_out=ACC[:, C_MASK:C_MASK + 1])

        # ---------------- huber big loop (per load chunk) ---------------
        # DVE: subtract + two 1-input TS ops (4x bf16 packed mode).
        # ScalarE: Square(clamp)+accum and Relu(e-1)+accum.
        # PE: ones^T x min(e,-1) 512-col slices accumulated into PSUM.
        negones_bf = sp.tile([P, 1], bf16)
        nc.vector.memset(negones_bf[:], -1.0)
        neg1 = sp.tile([P, 1], f32)
        nc.vector.memset(neg1[:], -1.0)
        pst = psp.tile([P, 512], f32)
        # PE accumulates sum(-m) over all chunks (single lhsT tile:
        # swapping ldweights inside one PSUM accumulation group was
        # observed to intermittently crash the exec unit).
        n_sl = sum((hi - lo) // 512 for lo, hi in bounds)
        sl_i = 0
        last = len(bounds) - 1
        for ci, (lo, hi) in enumerate(bounds):
            W = hi - lo
            e = epool.tile([P, W], bf16, tag="e", name=f"e{ci}")
            nc.vector.tensor_tensor(out=e[:], in0=pm[:, lo:hi],
                                    in1=tm[:, lo:hi], op=Alu.subtract)
            c = apool.tile([P, W], bf16, tag="c", name=f"c{ci}")
            nc.vector.tensor_scalar(out=c[:], in0=e[:], scalar1=-1.0,
                                    scalar2=1.0, op0=Alu.max, op1=Alu.min)
            m = spool.tile([P, W], bf16, tag="m", name=f"m{ci}")
            nc.vector.tensor_scalar(out=m[:], in0=e[:], scalar1=-1.0,
                                    scalar2=None, op0=Alu.min)
            d3 = dpool.tile([P, W], bf16, tag="d3", name=f"d3_{ci}")
            nc.scalar.activation(out=d3[:], in_=c[:], func=Act.Square,
                                 accum_out=ACC[:, C_C2 + ci:C_C2 + ci + 1])
            if ci > last - N_DVE_RELU:
                # DVE fused reduce: sum max(e,1) (host subtracts N*LAST_FRAC)
                d4 = dpool.tile([P, W], bf16, tag="d4", name=f"d4_{ci}")
                nc.vector.tensor_scalar(
                    out=d4[:], in0=e[:], scalar1=1.0, scalar2=0.0,
                    op0=Alu.max, op1=Alu.add,
                    accum_out=ACC[:, C_RP + ci:C_RP + ci + 1])
            else:
                d4 = dpool.tile([P, W], bf16, tag="d4", name=f"d4_{ci}")
                nc.scalar.activation(out=d4[:], in_=e[:], func=Act.Relu,
                                     bias=neg1[:],
                                     accum_out=ACC[:, C_RP + ci:C_RP + ci + 1])
            for sl in range(W // 512):
                nc.tensor.matmul(out=pst[0:1, :], lhsT=negones_bf[:],
                                 rhs=m[:, sl * 512:(sl + 1) * 512],
                                 start=(sl_i == 0), stop=(sl_i == n_sl - 1))
                sl_i += 1

        # ---------------- store (host does the partition sums) ---------
        nc.sync.dma_start(out=out_d[:, :], in_=ACC[:])
        msb = sp.tile([1, 512], f32)
        nc.scalar.copy(out=msb[:], in_=pst[0:1, :])
        nc.sync.dma_start(out=out2_d[:, :], in_=msb[:])
    nc.compile()
    return nc


_NC_CACHE = None


def _get_nc():
    global _NC_CACHE
    if _NC_CACHE is None:
        _NC_CACHE = build_nc()
    return _NC_CACHE


def _make_pack(inputs, lo, hi):
    """[128, PACKC] f32; row r = p*G + g maps to input row lo + r."""
    pk = np.empty((P, PACKC), dtype=np.float32)

    def blk(name):
        return np.ascontiguousarray(inputs[name][lo:hi]).reshape(P, G * K)

    pk[:, PC_V:PC_V + 48] = blk("cfs")
    pk[:, PC_V + 48:PC_V + 96] = blk("amps")
    pk[:, PC_V + 96:PC_V + 144] = blk("bws")
    pk[:, PC_GT:PC_GT + 48] = blk("gt_cfs")
    pk[:, PC_GT + 48:PC_GT + 96] = blk("gt_amps")
    pk[:, PC_GT + 96:PC_GT + 144] = blk("gt_bws")
    pk[:, PC_M:PC_M + 48] = blk("peak_mask")
    pk[:, PC_EXP:PC_EXP + G] = inputs["exponent"][lo:hi].reshape(P, G)
    pk[:, PC_EXP + G:PC_EXP + 2 * G] = inputs["offset"][lo:hi].reshape(P, G)
    pk[:, PC_GEXP:PC_GEXP + G] = inputs["gt_exponent"][lo:hi].reshape(P, G)
    pk[:, PC_GEXP + G:PC_GEXP + 2 * G] = inputs["gt_offset"][lo:hi].reshape(P, G)
    pk[:, PC_IOTA:PC_IOTA + G * K * K] = np.tile(
        (np.arange(K, dtype=np.float32) * 0.125), G * K)[None, :]
    pk[:, PC_ONES] = 1.0
    return pk


def combine(parts, pe_sum):
    """parts: [n_cores, ACC_COLS] f64.

    pe_sum = sum(-min(e,-1)) over ALL = sum relu(-e-1) + N.
    The last N_DVE_RELU C_RP cols hold sum max(e,1) = sum relu(e-1) + N*frac.
    """
    s = parts.sum(axis=0)
    n_big = float(B) * F
    S_c2 = s[C_C2:C_C2 + NCH].sum()    # sum clamp(e,-1,1)^2
    S_rp = s[C_RP:C_RP + NCH].sum() - n_big * LAST_FRAC
    huber_sum = 0.5 * S_c2 + S_rp + (pe_sum - n_big)
    l_recon = huber_sum / n_big
    SA = s[C_AMPS]
    l_sparse = SA / (B * K)
    l_bw = s[C_BW2] / (B * K)
    l_ap = s[C_AP] / B
    l_peaks = s[C_PK] / max(s[C_MASK], 1.0)
    SUN = SA + s[C_UMN]                # sum unm*amps (device has (unm-1)*amps)
    SUD = float(B) * K + s[C_UMD]      # sum unm     (device has unm-1 summed)
    l_um = SUN / max(SUD, 1.0)
    return (l_recon + 0.1 * l_sparse + 0.05 * l_bw + 0.5 * l_ap
            + 0.3 * l_peaks + 0.1 * l_um)


def run(inputs, **spmd_kwargs):
    nc = _get_nc()
    in_maps = []
    for c in range(N_CORES):
        lo, hi = c * BS, (c + 1) * BS
        in_maps.append({
            "pred_psd": np.ascontiguousarray(inputs["pred_psd"][lo:hi]),
            "true_psd": np.ascontiguousarray(inputs["true_psd"][lo:hi]),
            "pack": _make_pack(inputs, lo, hi),
        })
    res = run_bass_kernel_spmd(nc, in_maps, list(range(N_CORES)), **spmd_kwargs)
    parts = np.stack([r["out"].astype(np.float64).sum(axis=0)
                      for r in res.results])
    pe_sum = sum(float(r["out2"].astype(np.float64).sum()) for r in res.results)
    return np.float32(combine(parts, pe_sum)), res


def kernel(**inputs):
    out, _ = run(inputs)
    return out
